# revision 18
# baseline (speedup 1.0000x reference)
"""Trainium2 Bass kernel for nn_AttentionOutput (complex causal leaky-relu attention).

Reference (B=4, N=4096, F=64), per batch:
    sr = (Qr@Kr^T - Qi@Ki^T)/sqrt(N); si = (Qr@Ki^T + Qi@Kr^T)/sqrt(N)
    wr = tril * leaky_relu(sr);        wi = tril * leaky_relu(si)
    out_r = (wr@Vr)@W_att^T + b;       out_i = (wi@Vi)@W_att^T + b

Distribution: 2 cores per batch.  Core parity h processes j-blocks J === h
(mod 2) for ALL 4096 query rows; causal work is then identical across cores
(slot I needs 2I+2 j-blocks), so a single SPMD program serves all 8 cores and
the host sums the two partial outputs per batch.

Host-side layout prep removes every on-device transpose except the final
64x128 output transposes:
  - scores contract over p = f*2+c (128 partitions, ONE matmul per component):
    sr = Qmodr . K^T where Qmodr = Q with odd columns negated, and
    si = Qmodi . K^T where Qmodi = Q with column pairs swapped; K stays plain.
    Both Q variants are fed pre-transposed [128, N].
  - V' = (1/64) V @ W_att^T folds the score scale and the output projection
    into the attention-value matmul (leaky_relu is positively homogeneous);
    the bias is added on-device by the h=0 core only (h=1 gets a zero bias).
  - the causal triangle only affects each slot's last two j-blocks; those are
    masked with per-core [128,512] multiplicative masks fed from the host.
"""

import numpy as np

import concourse.bacc as bacc
import concourse.tile as tile
from concourse import mybir
from concourse.bass_utils import run_bass_kernel_spmd

B, N, F = 4, 4096, 64
P = 128             # = 2*F: score contraction width / partition count
JB = 128            # j-block width
IBW = 512           # i-block (slot) width
NSLOT = N // IBW    # 8 slots
NJPAR = N // JB // 2  # 16 parity j-blocks per core
NEG = 0.01
SCALE = 1.0 / 64.0  # 1/sqrt(N)
NCORES = 8

_DT = mybir.dt.float32
MM_BF16 = False     # bf16 matmul inputs: 4x PE throughput, half the DMA bytes

_CACHE: dict = {}
# leaky_relu lowering: True = ACT Copy(x*0.01) + DVE max (2 ops, CoreSim-validated,
# conservative); False = single ACT Lrelu (HW-validated standalone, not in CoreSim).
SIM_SAFE_LRELU = True


def _build_nc():
    nc = bacc.Bacc("TRN2", target_bir_lowering=False, num_devices=NCORES)
    dt = _DT
    mdt = mybir.dt.bfloat16 if MM_BF16 else _DT  # matmul input dtype
    qrT = nc.dram_tensor("qrT", [P, N], mdt, kind="ExternalInput")
    qiT = nc.dram_tensor("qiT", [P, N], mdt, kind="ExternalInput")
    kp = nc.dram_tensor("kp", [P, NJPAR * JB], mdt, kind="ExternalInput")
    vpr = nc.dram_tensor("vpr", [P, NJPAR * F], mdt, kind="ExternalInput")
    vpi = nc.dram_tensor("vpi", [P, NJPAR * F], mdt, kind="ExternalInput")
    dmask = nc.dram_tensor("dmask", [2, JB, IBW], mdt, kind="ExternalInput")
    # transposed output: rows 0:64 = y_r^T, rows 64:128 = y_i^T (host untransposes)
    out = nc.dram_tensor("out", [P, N], dt, kind="ExternalOutput")

    lrelu = mybir.ActivationFunctionType.Lrelu
    mul_op = mybir.AluOpType.mult

    with tile.TileContext(nc) as tc:
        with (
            tc.tile_pool(name="res", bufs=1) as res,
            tc.tile_pool(name="wp", bufs=4) as wp,
            tc.tile_pool(name="osb", bufs=2) as osb,
            tc.tile_pool(name="spsum", bufs=4, space="PSUM") as spsum,
            tc.tile_pool(name="ypsum", bufs=1, space="PSUM") as ypsum,
        ):
            sb_qr = res.tile([P, N], mdt, tag="qr")
            sb_qi = res.tile([P, N], mdt, tag="qi")
            for c in range(8):
                sl = slice(c * 512, (c + 1) * 512)
                nc.sync.dma_start(out=sb_qr[:, sl], in_=qrT[:, sl])
                nc.sync.dma_start(out=sb_qi[:, sl], in_=qiT[:, sl])
            sb_k = res.tile([P, NJPAR * JB], mdt, tag="k")
            for c in range(4):
                sl = slice(c * 512, (c + 1) * 512)
                nc.sync.dma_start(out=sb_k[:, sl], in_=kp[:, sl])
            sb_vr = res.tile([P, NJPAR * F], mdt, tag="vr")
            sb_vi = res.tile([P, NJPAR * F], mdt, tag="vi")
            for c in range(2):
                sl = slice(c * 512, (c + 1) * 512)
                nc.sync.dma_start(out=sb_vr[:, sl], in_=vpr[:, sl])
                nc.sync.dma_start(out=sb_vi[:, sl], in_=vpi[:, sl])
            sb_m0 = res.tile([JB, IBW], mdt, tag="m0")
            sb_m1 = res.tile([JB, IBW], mdt, tag="m1")
            nc.sync.dma_start(out=sb_m0, in_=dmask[0])
            nc.sync.dma_start(out=sb_m1, in_=dmask[1])
            sb_masks = (sb_m0, sb_m1)

            for s in range(NSLOT):
                cnt = 2 * s + 2
                isl = slice(s * IBW, (s + 1) * IBW)
                y_r = ypsum.tile([64, IBW], dt, tag="yr")
                y_i = ypsum.tile([64, IBW], dt, tag="yi")
                for p in range(cnt):
                    ksl = slice(p * JB, (p + 1) * JB)
                    vsl = slice(p * F, (p + 1) * F)
                    s_r = spsum.tile([JB, IBW], dt, tag="s")
                    nc.tensor.matmul(s_r[:], sb_k[:, ksl], sb_qr[:, isl],
                                     start=True, stop=True)
                    s_i = spsum.tile([JB, IBW], dt, tag="s")
                    nc.tensor.matmul(s_i[:], sb_k[:, ksl], sb_qi[:, isl],
                                     start=True, stop=True)
                    for s_ps, sb_v, y_ps in ((s_r, sb_vr, y_r), (s_i, sb_vi, y_i)):
                        w = wp.tile([JB, IBW], mdt, tag="w")
                        if SIM_SAFE_LRELU:
                            t = wp.tile([JB, IBW], mdt, tag="t")
                            nc.scalar.activation(
                                t[:], s_ps[:], mybir.ActivationFunctionType.Copy,
                                scale=NEG)
                            nc.vector.tensor_tensor(out=w[:], in0=s_ps[:], in1=t[:],
                                                    op=mybir.AluOpType.max)
                        else:
                            nc.scalar.activation(w[:], s_ps[:], lrelu, alpha=NEG)
                        if p >= cnt - 2:  # diagonal j-block: causal mask
                            wm = wp.tile([JB, IBW], mdt, tag="wm")
                            nc.vector.tensor_tensor(
                                out=wm[:], in0=w[:], in1=sb_masks[p - (cnt - 2)][:],
                                op=mul_op)
                            w = wm
                        nc.tensor.matmul(y_ps[:], sb_v[:, vsl], w[:],
                                         start=(p == 0), stop=(p == cnt - 1))
                # tail: accumulators to SBUF, then DMA out in transposed
                # layout; the host untransposes/interleaves and adds the bias.
                y_r_sb = osb.tile([64, IBW], dt, tag="ysbr")
                y_i_sb = osb.tile([64, IBW], dt, tag="ysbi")
                nc.scalar.copy(y_r_sb[:], y_r[:])
                nc.scalar.copy(y_i_sb[:], y_i[:])
                nc.sync.dma_start(out=out[0:64, isl], in_=y_r_sb[:])
                nc.sync.dma_start(out=out[64:128, isl], in_=y_i_sb[:])
    nc.compile()
    return nc


def _prep_inputs(Q, K, V, W_att, b_att):
    """Host-side re-layout: per-core in_maps for run_bass_kernel_spmd."""
    Q = np.asarray(Q, dtype=np.float32)
    K = np.asarray(K, dtype=np.float32)
    V = np.asarray(V, dtype=np.float32)
    W_att = np.asarray(W_att, dtype=np.float32)
    b_att = np.asarray(b_att, dtype=np.float32)

    Qf = Q.reshape(B, N, P)          # [b, i, f*2+c]
    Kf = K.reshape(B, N, P)
    Vpr = SCALE * (V[..., 0] @ W_att.T)   # [B, N, F]
    Vpi = SCALE * (V[..., 1] @ W_att.T)

    # causal masks for a slot's last two parity j-blocks, per core parity h:
    # block d = 2k+h (0-indexed from the slot's diagonal group of 4)
    jj = np.arange(JB)[:, None]
    ii = np.arange(IBW)[None, :]
    masks = {h: np.stack([(ii >= jj + JB * (2 * k + h)).astype(np.float32)
                          for k in range(2)]) for h in (0, 1)}

    in_maps = []
    for c in range(NCORES):
        b, h = divmod(c, 2)
        Qmodr = Qf[b].copy()
        Qmodr[:, 1::2] *= -1.0
        Qmodi = np.empty_like(Qf[b])
        Qmodi[:, 0::2] = Qf[b][:, 1::2]
        Qmodi[:, 1::2] = Qf[b][:, 0::2]
        kp = np.ascontiguousarray(
            Kf[b].reshape(N // JB, JB, P)[h::2].transpose(2, 0, 1).reshape(P, -1))
        vpr = np.ascontiguousarray(
            Vpr[b].reshape(N // JB, JB, F)[h::2].transpose(1, 0, 2).reshape(JB, -1))
        vpi = np.ascontiguousarray(
            Vpi[b].reshape(N // JB, JB, F)[h::2].transpose(1, 0, 2).reshape(JB, -1))
        m = {
            "qrT": np.ascontiguousarray(Qmodr.T),
            "qiT": np.ascontiguousarray(Qmodi.T),
            "kp": kp,
            "vpr": vpr,
            "vpi": vpi,
            "dmask": masks[h],
        }
        if MM_BF16:
            import ml_dtypes
            m = {k: v.astype(ml_dtypes.bfloat16) for k, v in m.items()}
        in_maps.append(m)
    return in_maps


def _gather(results, b_att):
    b_att = np.asarray(b_att, dtype=np.float32)
    out = np.empty((B, N, F, 2), dtype=np.float32)
    for b in range(B):
        y = results[2 * b]["out"] + results[2 * b + 1]["out"]  # [128, N] transposed
        out[b, :, :, 0] = y[0:64].T + b_att[None, :]
        out[b, :, :, 1] = y[64:128].T + b_att[None, :]
    return out


def kernel(Q, K, V, W_att, b_att):
    if "nc" not in _CACHE:
        _CACHE["nc"] = _build_nc()
    nc = _CACHE["nc"]
    in_maps = _prep_inputs(Q, K, V, W_att, b_att)
    res = run_bass_kernel_spmd(nc, in_maps, core_ids=list(range(NCORES)))
    return _gather(res.results, b_att)
